# revision 1
# baseline (speedup 1.0000x reference)
"""Trainium2 Bass kernel for ChebyshevActivation.

Math:
    scale = clip(input_scale, 0.1, 2.0)
    t = tanh(x * scale)                        # t in (-1, 1)
    out[b, o] = sum_w coeffs[o, w] * sum_i T_w(t[b, i])

Since |t| < 1, all Chebyshev T_n(t) lie in [-1, 1] and the reference's
clip(+-100) is dead code.  We work in the monomial basis: with power sums
M_j[b] = sum_i t[b,i]^j (M_0 = IN_F exactly) and G = coeffs @ C (C the
Chebyshev->monomial matrix), out = M @ G^T.

Power-sum extraction is one fused pass per "piece": ACT squares with
accum_out, or DVE scalar_tensor_tensor with accum_out (this environment's
walrus rejects TensorScalarPtr on GPSIMD and raw-ISA custom-DVE encodings
from plain Bass, so pieces live on ACT/DVE and the module is built with
Bacc).  Every moment can be split column-wise into pieces on different
engines; each piece accumulates into its own column of the per-tile moment
matrix, and the host duplicates the matching G rows so the final PE matmul
(K = #pieces+1) re-merges them.  Channels t1..t4 are fp16 (bf16 loses too
much precision through the basis change; fp16 keeps DVE 2x modes).

Per-core layout: data-parallel over batch, 8 cores x 1024 rows,
8 row-tiles of [128, 2048] per core.
"""

import numpy as np

import concourse.bass as bass
import concourse.bacc as bacc
import concourse.mybir as mybir
import concourse.tile as tile
from concourse import masks
from concourse.bass_utils import run_bass_kernel_spmd

# This environment's walrus build rejects raw client-encoded ISA instructions
# ("ISA wrong length" for the 64-byte EVENT_SEMAPHORE_RANGE_CLEAR emitted by
# the TileContext exit barrier).  Replace the range-clear with per-semaphore
# EventSemaphore writes (update_mode=sem-wr-imm, value 0), which this walrus
# accepts, so re-executing the loaded NEFF still sees cleared semaphores.
def _sem_clear_via_events(self, sem_range):
    # Spread the writes across all engines so they retire in parallel between
    # the two exit barriers instead of serially on GPSIMD.
    engines = list(self.bass.engines.values())
    inst = None
    for i, s in enumerate(sem_range):
        eng = engines[i % len(engines)]
        inst = mybir.InstEventSemaphore(
            name=self.bass.get_next_instruction_name(),
            ins=[], outs=[],
            sync_info=mybir.SyncInfo(
                on_wait=[],
                on_update=[mybir.SyncUpdate(
                    sync_type="semaphore", id=s,
                    update_mode="sem-wr-imm", update_value=0,
                )],
            ),
        )
        eng.add_instruction(inst)
    return inst


bass.BassGpSimd.sem_clear = _sem_clear_via_events

N_CORES = 8
BATCH = 8192
IN_F = 2048
OUT_F = 1024
DEG = 8
W = DEG + 1  # 9 moments
ROWS_PER_CORE = BATCH // N_CORES  # 1024
P = 128
NTILES = ROWS_PER_CORE // P  # 8

F32 = mybir.dt.float32
F16 = mybir.dt.float16
MULT = mybir.AluOpType.mult
ADD = mybir.AluOpType.add
SQUARE = mybir.ActivationFunctionType.Square
TANH = mybir.ActivationFunctionType.Tanh

# Stream definitions: name -> (in0, in1, value_dst) ; value_dst None => junk.
# in0 == in1 means the stream is a square (ACT-eligible).
STREAMS = {
    "t2": ("t1", "t1", "t2"),
    "t3": ("t2", "t1", "t3"),
    "t4": ("t2", "t2", "t4"),
    "M6": ("t3", "t3", None),
    "M5": ("t4", "t1", None),
    "M8": ("t4", "t4", None),
    "M7": ("t4", "t3", None),
}
STREAM_MOMENT = {"t2": 2, "t3": 3, "t4": 4, "M5": 5, "M6": 6, "M7": 7, "M8": 8}

# Engine assignment config: stream -> list of (engine, fraction).
# Engines: "A" = ACT square (squares only), "D" = DVE TTR, "G" = GPSIMD stt.
CFG = {
    "t2": [("D", 1.0)],
    "t3": [("D", 1.0)],
    "t4": [("A", 1.0)],
    "M5": [("D", 1.0)],
    "M6": [("A", 1.0)],
    "M7": [("D", 1.0)],
    "M8": [("A", 1.0)],
    "oc_act": 1.0,   # fraction of the PSUM->SBUF output copy done on ACT
    "mt": "D",       # moment-transpose PSUM->SBUF copy engine
    "xin_bufs": 4,
    "chan_bufs": 3,
    "t1_bufs": 3,
    "ostage_bufs": 3,
    # warm-up: chunk tile 0's tanh/t2/t3 into column halves (extra partial-
    # moment columns, re-merged by duplicated G rows) so DVE starts ~2us sooner
    "warm": True,
    "warm_set": ("t2", "t3"),
    # deprioritize the ACT output copy so the next tile's critical squares
    # (which DVE's cross-product streams wait on) win the scheduling race
    "oc_prio_bump": 32,
    # last tile: split the output copy across ACT+DVE halves with two
    # pipelined DMA-outs to shorten the tail chain
    "tail_fast": True,
}


def _cheb_monomial_matrix(deg=DEG):
    C = np.zeros((deg + 1, deg + 1), dtype=np.float64)
    C[0, 0] = 1.0
    if deg >= 1:
        C[1, 1] = 1.0
    for n in range(2, deg + 1):
        C[n, 1:] = 2.0 * C[n - 1, :-1]
        C[n, :] -= C[n - 2, :]
    return C


def _pieces(cfg):
    """Deterministic piece list: (stream, engine, col_lo, col_hi)."""
    out = []
    enabled = cfg.get("only_streams")
    for s in STREAMS:
        if enabled is not None and s not in enabled:
            continue
        cols = 0
        parts = cfg[s]
        for idx, (eng, frac) in enumerate(parts):
            if idx == len(parts) - 1:
                hi = IN_F
            else:
                hi = cols + int(round(IN_F * frac / 128.0)) * 128
                hi = min(hi, IN_F)
            if hi > cols:
                out.append((s, eng, cols, hi))
            cols = hi
    return out


def _moment_rows(cfg):
    """Row j of GT corresponds to these moments: [0 (M0), 1 (M1 tanh), *pieces,
    then warm-up duplicate rows for tile 0's chunked t1/t2/t3/t4 streams]."""
    rows = [0, 1]
    for s, _eng, _lo, _hi in _pieces(cfg):
        rows.append(STREAM_MOMENT[s])
    if cfg.get("warm"):
        rows += [1, 2, 3, 4]
    return rows



def _emit_out(nc, cfg, oc_pair, pout, ostage, mt_sb, gt_sb, out, it):
    if oc_pair:
        if not hasattr(nc, "_ocp_state") or it % 2 == 0:
            nc._ocp_state = (
                pout.tile([P, 2 * OUT_F], F32, tag="opp"),
                ostage.tile([P, 2 * OUT_F], F32, tag="osp"),
            )
        o_ps_pair, o_sb_pair = nc._ocp_state
        base = (it % 2) * OUT_F
        for h in range(2):
            nc.tensor.matmul(
                o_ps_pair[:, base + h * 512:base + (h + 1) * 512],
                lhsT=mt_sb[:, :],
                rhs=gt_sb[:, h * 512:(h + 1) * 512],
                start=True, stop=True,
            )
        if it % 2 == 1:
            nc.scalar.copy(o_sb_pair[:, :], o_ps_pair[:, :])
            for s in range(2):
                it0 = it - 1 + s
                nc.sync.dma_start(
                    out=out[it0 * P:(it0 + 1) * P, :],
                    in_=o_sb_pair[:, s * OUT_F:(s + 1) * OUT_F],
                )
    else:
        o_ps = pout.tile([P, OUT_F], F32)
        for h in range(2):
            nc.tensor.matmul(
                o_ps[:, h * 512:(h + 1) * 512],
                lhsT=mt_sb[:, :],
                rhs=gt_sb[:, h * 512:(h + 1) * 512],
                start=True, stop=True,
            )
        o_sb = ostage.tile([P, OUT_F], F32)
        if cfg.get("tail_fast") and it == NTILES - 1:
            # split the last tile's output copy across ACT+DVE in parallel
            # halves, each followed by its own DMA, to shorten the tail chain
            H2 = OUT_F // 2
            nc.scalar.copy(o_sb[:, 0:H2], o_ps[:, 0:H2])
            nc.vector.tensor_copy(o_sb[:, H2:OUT_F], o_ps[:, H2:OUT_F])
            nc.sync.dma_start(out=out[it * P:(it + 1) * P, 0:H2],
                              in_=o_sb[:, 0:H2])
            nc.sync.dma_start(out=out[it * P:(it + 1) * P, H2:OUT_F],
                              in_=o_sb[:, H2:OUT_F])
            return
        ca = int(round(OUT_F * cfg["oc_act"] / 128.0)) * 128
        ca = max(0, min(OUT_F, ca))
        if cfg.get("oc_last_dve") and it == NTILES - 1:
            ca = 0
        ocb = cfg.get("oc_prio_bump", 0)
        if ca > 0:
            r = nc.scalar.copy(o_sb[:, 0:ca], o_ps[:, 0:ca])
            if ocb:
                r.ins.bass_priority += ocb
        if ca < OUT_F:
            r = nc.vector.tensor_copy(o_sb[:, ca:OUT_F], o_ps[:, ca:OUT_F])
            if ocb:
                r.ins.bass_priority += ocb
        nc.sync.dma_start(out=out[it * P:(it + 1) * P, :], in_=o_sb[:, :])


def _build_nc(scale: float, cfg=CFG) -> bass.Bass:
    pieces = _pieces(cfg)
    warm = bool(cfg.get("warm"))
    K = 2 + len(pieces) + (4 if warm else 0)  # M0 + M1 + pieces [+ warm dups]
    assert K <= 24
    mcols = K

    nc = bacc.Bacc("TRN2")
    x = nc.dram_tensor("x", [ROWS_PER_CORE, IN_F], F32, kind="ExternalInput")
    gt = nc.dram_tensor("gt", [K, OUT_F], F32, kind="ExternalInput")
    out = nc.dram_tensor("out", [ROWS_PER_CORE, OUT_F], F32, kind="ExternalOutput")

    oc_pair = cfg.get("oc_pair", False)
    with tile.TileContext(nc) as tc:
        with (
            tc.tile_pool(name="singles", bufs=1) as singles,
            tc.tile_pool(name="xin", bufs=cfg["xin_bufs"]) as xin,
            tc.tile_pool(name="chan", bufs=cfg["chan_bufs"]) as chan,
            tc.tile_pool(name="chan1", bufs=cfg.get("t1_bufs", cfg["chan_bufs"])) as chan1,
            tc.tile_pool(name="junk", bufs=1) as junkp,
            tc.tile_pool(name="mpool", bufs=4) as mpool,
            tc.tile_pool(name="mtsb", bufs=4) as mtsb,
            tc.tile_pool(name="ostage", bufs=cfg["ostage_bufs"]) as ostage,
            tc.tile_pool(name="pt", bufs=cfg.get("pt_bufs", 2), space="PSUM") as pt,
            tc.tile_pool(name="pout", bufs=(1 if oc_pair else cfg.get("pout_bufs", 2)),
                         space="PSUM") as pout,
        ):
            if cfg.get("mt_batch", 1) > 1:
                gt_sb = singles.tile([32 + K, OUT_F], F32)
                nc.sync.dma_start(out=gt_sb[0:K, :], in_=gt[:, :])
                nc.sync.dma_start(out=gt_sb[32:32 + K, :], in_=gt[:, :])
            else:
                gt_sb = singles.tile([K, OUT_F], F32)
                nc.sync.dma_start(out=gt_sb[:, :], in_=gt[:, :])
            ident = singles.tile([P, P], F32)
            masks.make_identity(nc, ident[:, :])

            j_dve = junkp.tile([P, IN_F], F16, tag="jd")
            j_act = junkp.tile([P, IN_F], F16, tag="ja")
            j_gps = junkp.tile([P, IN_F], F16, tag="jg")
            JUNK = {"A": j_act, "D": j_dve, "G": j_gps}

            mt_batch = cfg.get("mt_batch", 1)
            m_pair = None
            for it in range(NTILES):
                x_t = xin.tile([P, IN_F], F32)
                chunked = warm and it == 0
                H = IN_F // 2
                if chunked:
                    nc.sync.dma_start(out=x_t[:, 0:H], in_=x[it * P:(it + 1) * P, 0:H])
                    nc.sync.dma_start(out=x_t[:, H:IN_F], in_=x[it * P:(it + 1) * P, H:IN_F])
                else:
                    nc.sync.dma_start(out=x_t[:, :], in_=x[it * P:(it + 1) * P, :])

                if mt_batch > 1:
                    if it % mt_batch == 0:
                        m_pair = mpool.tile([P, mt_batch * 32], F32, tag="mp")
                    m_t = m_pair[:, (it % mt_batch) * 32:(it % mt_batch) * 32 + mcols]
                else:
                    m_t = mpool.tile([P, mcols], F32)
                nc.gpsimd.memset(m_t[:, 0:1], float(IN_F))

                t1 = chan1.tile([P, IN_F], F16, tag="t1")
                t2 = chan.tile([P, IN_F], F16, tag="t2")
                t3 = chan.tile([P, IN_F], F16, tag="t3")
                t4 = chan.tile([P, IN_F], F16, tag="t4")
                VALS = {"t1": t1, "t2": t2, "t3": t3, "t4": t4}

                if warm and not chunked:
                    nc.gpsimd.memset(m_t[:, K - 4:K], 0.0)
                elif warm and chunked:
                    # zero warm columns whose stream is not chunked on tile 0
                    wset = cfg.get("warm_set", ("t2", "t3", "t4"))
                    for nm, off in (("t2", 1), ("t3", 2), ("t4", 3)):
                        if nm not in wset:
                            nc.gpsimd.memset(m_t[:, K - 4 + off:K - 3 + off], 0.0)

                # t1 = tanh(scale * x), accum -> M1 (col 1; chunk b -> warm col)
                if chunked:
                    nc.scalar.activation(
                        out=t1[:, 0:H], in_=x_t[:, 0:H], func=TANH,
                        scale=scale, accum_out=m_t[:, 1:2],
                    )
                    nc.scalar.activation(
                        out=t1[:, H:IN_F], in_=x_t[:, H:IN_F], func=TANH,
                        scale=scale, accum_out=m_t[:, K - 4:K - 3],
                    )
                else:
                    nc.scalar.activation(
                        out=t1[:, :], in_=x_t[:, :], func=TANH,
                        scale=scale, accum_out=m_t[:, 1:2],
                    )

                for pidx, (s, eng, lo, hi) in enumerate(pieces):
                    a_name, b_name, dst_name = STREAMS[s]
                    a = VALS[a_name]
                    b = VALS[b_name]
                    dst = VALS[dst_name] if dst_name else JUNK[eng]
                    mcol = m_t[:, 2 + pidx:3 + pidx]
                    if (chunked and s in cfg.get("warm_set", ("t2", "t3", "t4"))
                            and lo == 0 and hi == IN_F):
                        # split tile-0 value streams; 2nd chunk accums into warm col
                        wcol_i = K - 4 + {"t2": 1, "t3": 2, "t4": 3}[s]
                        wcol = m_t[:, wcol_i:wcol_i + 1]
                        for (clo, chi, mc) in ((0, H, mcol), (H, IN_F, wcol)):
                            if eng == "A":
                                nc.scalar.activation(
                                    out=dst[:, clo:chi], in_=a[:, clo:chi],
                                    func=SQUARE, accum_out=mc,
                                )
                            else:
                                nc.vector.scalar_tensor_tensor(
                                    out=dst[:, clo:chi], in0=a[:, clo:chi],
                                    scalar=1.0, in1=b[:, clo:chi],
                                    op0=MULT, op1=MULT, accum_out=mc,
                                )
                        continue
                    if eng == "A":
                        assert a_name == b_name, (s, "ACT needs a square")
                        nc.scalar.activation(
                            out=dst[:, lo:hi], in_=a[:, lo:hi], func=SQUARE,
                            accum_out=mcol,
                        )
                    elif eng == "D":
                        nc.vector.scalar_tensor_tensor(
                            out=dst[:, lo:hi], in0=a[:, lo:hi], scalar=1.0,
                            in1=b[:, lo:hi], op0=MULT, op1=MULT,
                            accum_out=mcol,
                        )
                    elif eng == "G":
                        nc.gpsimd.scalar_tensor_tensor(
                            out=dst[:, lo:hi], in0=a[:, lo:hi], scalar=1.0,
                            in1=b[:, lo:hi], op0=MULT, op1=MULT,
                            accum_out=mcol,
                        )
                    else:
                        raise ValueError(eng)

                # Transpose moments: [128, K*] -> [K*, 128] PSUM, copy to SBUF
                if mt_batch > 1:
                    if it % mt_batch != mt_batch - 1:
                        continue_tail = True
                    mt_rows = None
                    if it % mt_batch == mt_batch - 1:
                        mt_ps = pt.tile([mt_batch * 32, P], F32, tag="mtp")
                        nc.tensor.transpose(mt_ps[:, :], m_pair[:, :], ident[:, :])
                        mt_all = mtsb.tile([mt_batch * 32, P], F32, tag="mta")
                        if cfg["mt"] == "D":
                            nc.vector.tensor_copy(mt_all[:, :], mt_ps[:, :])
                        else:
                            nc.scalar.copy(mt_all[:, :], mt_ps[:, :])
                    else:
                        continue
                else:
                    mt_ps = pt.tile([mcols, P], F32)
                    nc.tensor.transpose(mt_ps[:, :], m_t[:, :], ident[:, :])
                    mt_sb = mtsb.tile([mcols, P], F32)
                    if cfg["mt"] == "D":
                        r = nc.vector.tensor_copy(mt_sb[:, :], mt_ps[:, :])
                    else:
                        r = nc.scalar.copy(mt_sb[:, :], mt_ps[:, :])
                    if cfg.get("mt_prio_bump", 0):
                        r.ins.bass_priority += cfg["mt_prio_bump"]

                # out[128, 1024] = MT.T @ GT  (contraction K)
                sub_tiles = ([it] if cfg.get("mt_batch", 1) == 1 else
                             list(range(it - cfg["mt_batch"] + 1, it + 1)))
                for sit in sub_tiles:
                    if cfg.get("mt_batch", 1) > 1:
                        sidx = sit - (it - cfg["mt_batch"] + 1)
                        mt_sb = mt_all[sidx * 32:sidx * 32 + mcols, :]
                        gt_use = gt_sb[sidx * 32:sidx * 32 + mcols, :]
                    else:
                        gt_use = gt_sb[:, :]
                    _emit_out(nc, cfg, oc_pair, pout, ostage, mt_sb, gt_use, out, sit)

    nc.finalize()
    return nc


_NC_CACHE: dict[tuple, bass.Bass] = {}


def _host_gt(coeffs, cfg=CFG):
    C = _cheb_monomial_matrix()
    G = (coeffs.astype(np.float64) @ C).astype(np.float32)  # [OUT_F, W]
    rows = _moment_rows(cfg)
    GT = np.ascontiguousarray(G.T[rows, :])  # [K, OUT_F]
    return GT


def _run(x, coeffs, input_scale, cfg=CFG, **spmd_kwargs):
    x = np.ascontiguousarray(np.asarray(x, dtype=np.float32))
    coeffs = np.asarray(coeffs, dtype=np.float32)
    scale = float(np.clip(np.asarray(input_scale, dtype=np.float32), 0.1, 2.0).reshape(-1)[0])

    GT = _host_gt(coeffs, cfg)

    key = (scale, str(cfg))
    nc = _NC_CACHE.get(key)
    if nc is None:
        nc = _build_nc(scale, cfg)
        _NC_CACHE[key] = nc

    in_maps = [
        {"x": np.ascontiguousarray(x[c * ROWS_PER_CORE:(c + 1) * ROWS_PER_CORE]),
         "gt": GT}
        for c in range(N_CORES)
    ]
    res = run_bass_kernel_spmd(nc, in_maps, core_ids=list(range(N_CORES)), **spmd_kwargs)
    out = np.concatenate([res.results[c]["out"] for c in range(N_CORES)], axis=0)
    return out.astype(np.float32), res


def kernel(x, coeffs, input_scale):
    out, _ = _run(x, coeffs, input_scale)
    return out


if __name__ == "__main__":
    rng = np.random.default_rng(0)
    x = rng.standard_normal((BATCH, IN_F), dtype=np.float32)
    coeffs = (rng.standard_normal((OUT_F, W)) * 0.1).astype(np.float32)
    s = np.ones((1,), np.float32)
    out = kernel(x=x, coeffs=coeffs, input_scale=s)
    print(out.shape, out.dtype)



# revision 17
# speedup vs baseline: 1.0781x; 1.0781x over previous
"""Trainium2 Bass kernel for ChebyshevActivation.

Math:
    scale = clip(input_scale, 0.1, 2.0)
    t = tanh(x * scale)                        # t in (-1, 1)
    out[b, o] = sum_w coeffs[o, w] * sum_i T_w(t[b, i])

Since |t| < 1 the reference's clip(+-100) is dead code.  We work in the
monomial basis: with power sums M_j[b] = sum_i t[b,i]^j (M_0 = IN_F
exactly) and G = coeffs @ C (C the Chebyshev->monomial matrix),
out = M @ G^T.

Op selection is driven by the cost model's DVE fast-mode table:
  - InstTensorScalarPtr with TWO tensor operands (scalar_tensor_tensor)
    runs 1x, but the scalar-immediate form (tensor_scalar, two-op variant
    with accum_out) runs 4x on fp16 -> a full-row moment extraction is
    ~533ns instead of ~2133ns.
  - tensor_tensor (mult) runs 2x on fp16 -> product streams at ~1067ns.
  - ACT activations (tanh/square) are 1x but a parallel engine, and their
    fused accum_out gives the stream's moment for ~187ns extra.
  - GPSIMD tensor_tensor (walrus-accepted here) adds a third elementwise
    engine at ~2.03ns/col for junk product streams.

Streams: t1=tanh(x), t2=t1^2 (ACT), t3=t2*t1 (DVE), t4=t2^2 (ACT);
junk products t5=t4*t1, t6=t3^2, t7=t4*t3, t8=t4^2 carry M5..M8.  Every
junk stream's columns can be split ACT(square-only)/GPS/DVE per CFG; each
ACT piece accumulates its own m-column, the GPS+DVE columns land in one
junk tile read by a single 4x tensor_scalar accum.  The host duplicates
the matching G rows so the final PE matmul re-merges all pieces.

Per-core layout: data-parallel over batch, 8 cores x 1024 rows,
8 row-tiles of [128, 2048] per core.
"""

import numpy as np

import concourse.bass as bass
import concourse.bacc as bacc
import concourse.mybir as mybir
import concourse.tile as tile
from concourse import masks
from concourse.bass_utils import run_bass_kernel_spmd

# This environment's walrus build rejects raw client-encoded ISA instructions
# ("ISA wrong length" for the 64-byte EVENT_SEMAPHORE_RANGE_CLEAR emitted by
# the TileContext exit barrier).  Replace the range-clear with per-semaphore
# EventSemaphore writes (update_mode=sem-wr-imm, value 0), which this walrus
# accepts, so re-executing the loaded NEFF still sees cleared semaphores.
def _sem_clear_via_events(self, sem_range):
    engines = list(self.bass.engines.values())
    inst = None
    for i, s in enumerate(sem_range):
        eng = engines[i % len(engines)]
        inst = mybir.InstEventSemaphore(
            name=self.bass.get_next_instruction_name(),
            ins=[], outs=[],
            sync_info=mybir.SyncInfo(
                on_wait=[],
                on_update=[mybir.SyncUpdate(
                    sync_type="semaphore", id=s,
                    update_mode="sem-wr-imm", update_value=0,
                )],
            ),
        )
        eng.add_instruction(inst)
    return inst


bass.BassGpSimd.sem_clear = _sem_clear_via_events

N_CORES = 8
BATCH = 8192
IN_F = 2048
OUT_F = 1024
DEG = 8
W = DEG + 1
ROWS_PER_CORE = BATCH // N_CORES  # 1024
P = 128
NTILES = ROWS_PER_CORE // P  # 8

F32 = mybir.dt.float32
F16 = mybir.dt.float16
MULT = mybir.AluOpType.mult
ADD = mybir.AluOpType.add
SQUARE = mybir.ActivationFunctionType.Square
TANH = mybir.ActivationFunctionType.Tanh

# Column splits per junk stream s: (act_cols, gps_cols); DVE takes the rest.
# act pieces are squares so only t6 (=t3^2) and t8 (=t4^2) can use ACT.
CFG = {
    "a2": IN_F,      # ACT cols of t2 square (value stream)
    "a4": IN_F,      # ACT cols of t4 square (value stream)
    "a6": 1024,      # ACT cols of t6 (M6)
    "a8": 0,         # ACT cols of t8 (M8)
    "g5": 2048,      # GPS cols of t5 (M5)
    "g7": 1024,      # GPS cols of t7 (M7)
    "g8": 0,         # GPS cols of t8 (M8)
    "oc_act": 0,     # out-copy cols on ACT (rest DVE)
    "warm": True,    # split tile 0's x DMA + tanh into halves
    "tail_delay": 2,  # software-pipeline depth for the per-tile tail
    "xin_bufs": 3,
    "val_bufs": 3,
    "junk_bufs": 3,
    "ostage_bufs": 3,
    "tail_fast": True,
}


def _cheb_monomial_matrix(deg=DEG):
    C = np.zeros((deg + 1, deg + 1), dtype=np.float64)
    C[0, 0] = 1.0
    if deg >= 1:
        C[1, 1] = 1.0
    for n in range(2, deg + 1):
        C[n, 1:] = 2.0 * C[n - 1, :-1]
        C[n, :] -= C[n - 2, :]
    return C


def _plan(cfg):
    """Ordered m-column list: (moment_k, tag). Single source of truth for
    both the kernel emission and the host GT row duplication."""
    cols = [(0, "M0"), (1, "M1")]
    if cfg["a2"] > 0:
        cols.append((2, "M2a"))
    if cfg["a2"] < IN_F:
        cols.append((2, "M2d"))
    if cfg["a4"] > 0:
        cols.append((4, "M4a"))
    if cfg["a4"] < IN_F:
        cols.append((4, "M4d"))
    cols.append((3, "M3"))
    for k, a_key, g_key in ((5, None, "g5"), (6, "a6", None),
                            (7, None, "g7"), (8, "a8", "g8")):
        a = cfg.get(a_key, 0) if a_key else 0
        g = cfg.get(g_key, 0) if g_key else 0
        if a > 0:
            cols.append((k, f"M{k}a"))
        if a + g < IN_F or g > 0:
            cols.append((k, f"M{k}d"))
    if cfg.get("warm"):
        cols.append((1, "W1"))
    return cols


def _build_nc(scale: float, cfg=CFG) -> bass.Bass:
    plan = _plan(cfg)
    K = len(plan)
    assert K <= 32
    idx = {tag: i for i, (_k, tag) in enumerate(plan)}

    nc = bacc.Bacc("TRN2")
    x = nc.dram_tensor("x", [ROWS_PER_CORE, IN_F], F32, kind="ExternalInput")
    gt = nc.dram_tensor("gt", [K, OUT_F], F32, kind="ExternalInput")
    out = nc.dram_tensor("out", [ROWS_PER_CORE, OUT_F], F32,
                         kind="ExternalOutput")

    with tile.TileContext(nc) as tc:
        with (
            tc.tile_pool(name="singles", bufs=1) as singles,
            tc.tile_pool(name="xin", bufs=cfg["xin_bufs"]) as xin,
            tc.tile_pool(name="vals", bufs=cfg["val_bufs"]) as vals,
            tc.tile_pool(name="junk", bufs=cfg["junk_bufs"]) as junkp,
            tc.tile_pool(name="janx", bufs=2) as janx,
            tc.tile_pool(name="mpool", bufs=4) as mpool,
            tc.tile_pool(name="mtsb", bufs=4) as mtsb,
            tc.tile_pool(name="ostage", bufs=cfg["ostage_bufs"]) as ostage,
            tc.tile_pool(name="pt", bufs=2, space="PSUM") as pt,
            tc.tile_pool(name="pout", bufs=2, space="PSUM") as pout,
        ):
            gt_sb = singles.tile([K, OUT_F], F32)
            ident = singles.tile([P, P], F32)
            if not cfg.get("gt_late"):
                nc.sync.dma_start(out=gt_sb[:, :], in_=gt[:, :])
                masks.make_identity(nc, ident[:, :])

            def front(it):
                """Tile front: DMA, tanh, value squares, product streams.
                Returns the closure state for the deferred tail."""
                r0 = it * P
                x_t = xin.tile([P, IN_F], F32)
                chunked = cfg.get("warm") and it == 0
                H = IN_F // 2
                if chunked:
                    nc.sync.dma_start(out=x_t[:, 0:H], in_=x[r0:r0 + P, 0:H])
                    nc.sync.dma_start(out=x_t[:, H:IN_F],
                                      in_=x[r0:r0 + P, H:IN_F])
                else:
                    nc.sync.dma_start(out=x_t[:, :], in_=x[r0:r0 + P, :])
                if it == 0 and cfg.get("gt_late"):
                    # gt + identity after tile 0's x so tanh starts sooner
                    nc.sync.dma_start(out=gt_sb[:, :], in_=gt[:, :])
                    masks.make_identity(nc, ident[:, :])

                m_t = mpool.tile([P, K], F32)
                nc.gpsimd.memset(m_t[:, 0:1], float(IN_F))
                if cfg.get("warm") and not chunked:
                    w = idx["W1"]
                    nc.gpsimd.memset(m_t[:, w:w + 1], 0.0)

                def mcol(tag):
                    i = idx[tag]
                    return m_t[:, i:i + 1]

                t1 = vals.tile([P, IN_F], F16, tag="t1")
                t2 = vals.tile([P, IN_F], F16, tag="t2")
                t3 = vals.tile([P, IN_F], F16, tag="t3")
                t4 = vals.tile([P, IN_F], F16, tag="t4")

                # t1 = tanh(scale*x), accum -> M1 (warm: halves on tile 0)
                if chunked:
                    nc.scalar.activation(out=t1[:, 0:H], in_=x_t[:, 0:H],
                                         func=TANH, scale=scale,
                                         accum_out=mcol("M1"))
                    nc.scalar.activation(out=t1[:, H:IN_F], in_=x_t[:, H:IN_F],
                                         func=TANH, scale=scale,
                                         accum_out=mcol("W1"))
                else:
                    nc.scalar.activation(out=t1[:, :], in_=x_t[:, :],
                                         func=TANH, scale=scale,
                                         accum_out=mcol("M1"))

                def val_square(src, dst, a_cols, tag_a, tag_d):
                    """Value stream dst = src^2: ACT [0:a] + DVE [a:IN_F]."""
                    if a_cols > 0:
                        nc.scalar.activation(out=dst[:, 0:a_cols],
                                             in_=src[:, 0:a_cols], func=SQUARE,
                                             accum_out=mcol(tag_a))
                    if a_cols < IN_F:
                        nc.vector.tensor_tensor(out=dst[:, a_cols:IN_F],
                                                in0=src[:, a_cols:IN_F],
                                                in1=src[:, a_cols:IN_F],
                                                op=MULT)
                        nc.vector.tensor_scalar(out=dst[:, a_cols:IN_F],
                                                in0=dst[:, a_cols:IN_F],
                                                scalar1=1.0, scalar2=0.0,
                                                op0=MULT, op1=ADD,
                                                accum_out=mcol(tag_d))

                val_square(t1, t2, cfg["a2"], "M2a", "M2d")

                # t3 = t2*t1 (DVE 2x) + M3 via 4x accum into a scratch dump
                # (NOT in-place: an in-place ts would add a false write dep
                # serializing every downstream reader of t3 behind it).
                nc.vector.tensor_tensor(out=t3[:, :], in0=t2[:, :],
                                        in1=t1[:, :], op=MULT)
                tsd = junkp.tile([P, IN_F], F16, tag="tsd")
                nc.vector.tensor_scalar(out=tsd[:, :], in0=t3[:, :],
                                        scalar1=1.0, scalar2=0.0,
                                        op0=MULT, op1=ADD,
                                        accum_out=mcol("M3"))

                val_square(t2, t4, cfg["a4"], "M4a", "M4d")

                deferred = []
                jd_of = {}

                def junk_moment(k, in0, in1, a_cols, g_cols, gps_ins=None):
                    """Junk product stream: ACT square [0:a] (own accum,
                    deferred to the tail since it may wait on DVE's t3),
                    GPS tt [a:a+g], DVE tt [a+g:IN_F]; one 4x ts accum over
                    [a:IN_F], deferred when fed by the slow GPSIMD engine so
                    the in-order DVE queue never stalls on it."""
                    if a_cols > 0:
                        def emit_sq(k=k, in0=in0, a_cols=a_cols):
                            ja = janx.tile([P, a_cols], F16, tag=f"ja{k}")
                            nc.scalar.activation(out=ja[:, :],
                                                 in_=in0[:, 0:a_cols],
                                                 func=SQUARE,
                                                 accum_out=mcol(f"M{k}a"))
                        deferred.append(emit_sq)
                    lo = a_cols
                    mid = min(IN_F, a_cols + g_cols)
                    if lo >= IN_F:
                        return
                    jd = junkp.tile([P, IN_F - lo], F16, tag=f"jd{k}")
                    jd_of[k] = jd
                    if mid > lo:
                        gi0, gi1 = gps_ins if gps_ins else (in0, in1)
                        nc.gpsimd.tensor_tensor(out=jd[:, 0:mid - lo],
                                                in0=gi0[:, lo:mid],
                                                in1=gi1[:, lo:mid], op=MULT)
                    if mid < IN_F:
                        nc.vector.tensor_tensor(out=jd[:, mid - lo:],
                                                in0=in0[:, mid:IN_F],
                                                in1=in1[:, mid:IN_F], op=MULT)

                    def emit_ts(jd=jd, k=k):
                        nc.vector.tensor_scalar(out=jd[:, :], in0=jd[:, :],
                                                scalar1=1.0, scalar2=0.0,
                                                op0=MULT, op1=ADD,
                                                accum_out=mcol(f"M{k}d"))
                    if mid > lo:
                        deferred.append(emit_ts)
                    else:
                        emit_ts()

                last = it == NTILES - 1 and cfg.get("last_no_gps")
                g5 = 0 if last else cfg["g5"]
                g7 = 0 if last else cfg["g7"]
                g8 = 0 if last else cfg["g8"]
                if cfg.get("gps_chain"):
                    # GPS streams avoid t4 (the longest ACT chain): t5 = t2*t3
                    # and t7 = jd5*t2 (Pool self-chain, valid if g7 <= g5).
                    assert g7 <= g5 or g5 == 0
                    junk_moment(5, t2, t3, 0, g5)
                    junk_moment(8, t4, t4, cfg["a8"], g8)
                    junk_moment(6, t3, t3, cfg["a6"], 0)
                    junk_moment(7, t4, t3, 0, g7,
                                gps_ins=(jd_of[5], t2) if g5 else None)
                else:
                    junk_moment(8, t4, t4, cfg["a8"], g8)
                    junk_moment(6, t3, t3, cfg["a6"], 0)
                    junk_moment(5, t4, t1, 0, g5)
                    junk_moment(7, t4, t3, 0, g7)
                return {"it": it, "r0": r0, "m_t": m_t, "deferred": deferred}

            def tail(st):
                """Tile tail, emitted one iteration later so its cross-engine
                waits overlap the next tile's front work."""
                it, r0, m_t = st["it"], st["r0"], st["m_t"]
                for emit in st["deferred"]:
                    emit()

                # Transpose moments [P, K] -> [K, P] PSUM, copy to SBUF
                mt_ps = pt.tile([K, P], F32)
                nc.tensor.transpose(mt_ps[:, :], m_t[:, :], ident[:, :])
                mt_sb = mtsb.tile([K, P], F32)
                nc.vector.tensor_copy(mt_sb[:, :], mt_ps[:, :])

                # out[128, 1024] = MT.T @ GT  (contraction K)
                o_ps = pout.tile([P, OUT_F], F32)
                for h in range(2):
                    nc.tensor.matmul(o_ps[:, h * 512:(h + 1) * 512],
                                     lhsT=mt_sb[:, :],
                                     rhs=gt_sb[:, h * 512:(h + 1) * 512],
                                     start=True, stop=True)
                o_sb = ostage.tile([P, OUT_F], F32)
                ca = cfg["oc_act"]
                if cfg.get("tail_fast") and it == NTILES - 1:
                    H2 = OUT_F // 2
                    nc.scalar.copy(o_sb[:, 0:H2], o_ps[:, 0:H2])
                    nc.vector.tensor_copy(o_sb[:, H2:OUT_F], o_ps[:, H2:OUT_F])
                    nc.sync.dma_start(out=out[r0:r0 + P, 0:H2],
                                      in_=o_sb[:, 0:H2])
                    nc.sync.dma_start(out=out[r0:r0 + P, H2:OUT_F],
                                      in_=o_sb[:, H2:OUT_F])
                    return
                if ca > 0:
                    nc.scalar.copy(o_sb[:, 0:ca], o_ps[:, 0:ca])
                if ca < OUT_F:
                    nc.vector.tensor_copy(o_sb[:, ca:OUT_F], o_ps[:, ca:OUT_F])
                nc.sync.dma_start(out=out[r0:r0 + P, :], in_=o_sb[:, :])

            pending = []
            D = cfg.get("tail_delay", 1)
            for it in range(NTILES):
                st = front(it)
                pending.append(st)
                if len(pending) > D:
                    tail(pending.pop(0))
            for st in pending:
                tail(st)

    nc.finalize()
    return nc


_NC_CACHE: dict[tuple, bass.Bass] = {}


def _host_gt(coeffs, cfg=CFG):
    C = _cheb_monomial_matrix()
    G = (coeffs.astype(np.float64) @ C).astype(np.float32)  # [OUT_F, W]
    rows = [k for k, _tag in _plan(cfg)]
    GT = np.ascontiguousarray(G.T[rows, :])  # [K, OUT_F]
    return GT


def _run(x, coeffs, input_scale, cfg=CFG, **spmd_kwargs):
    x = np.ascontiguousarray(np.asarray(x, dtype=np.float32))
    coeffs = np.asarray(coeffs, dtype=np.float32)
    scale = float(np.clip(np.asarray(input_scale, dtype=np.float32),
                          0.1, 2.0).reshape(-1)[0])

    GT = _host_gt(coeffs, cfg)

    key = (scale, str(cfg))
    nc = _NC_CACHE.get(key)
    if nc is None:
        nc = _build_nc(scale, cfg)
        _NC_CACHE[key] = nc

    in_maps = [
        {"x": np.ascontiguousarray(x[c * ROWS_PER_CORE:(c + 1) * ROWS_PER_CORE]),
         "gt": GT}
        for c in range(N_CORES)
    ]
    res = run_bass_kernel_spmd(nc, in_maps, core_ids=list(range(N_CORES)),
                               **spmd_kwargs)
    out = np.concatenate([res.results[c]["out"] for c in range(N_CORES)],
                         axis=0)
    return out.astype(np.float32), res


def kernel(x, coeffs, input_scale):
    out, _ = _run(x, coeffs, input_scale)
    return out


if __name__ == "__main__":
    rng = np.random.default_rng(0)
    x = rng.standard_normal((BATCH, IN_F), dtype=np.float32)
    coeffs = (rng.standard_normal((OUT_F, W)) * 0.1).astype(np.float32)
    s = np.ones((1,), np.float32)
    out = kernel(x=x, coeffs=coeffs, input_scale=s)
    print(out.shape, out.dtype)


# revision 23
# speedup vs baseline: 1.1299x; 1.0481x over previous
"""Trainium2 Bass kernel for ChebyshevActivation.

Math:
    scale = clip(input_scale, 0.1, 2.0)
    t = tanh(x * scale)                        # t in (-1, 1)
    out[b, o] = sum_w coeffs[o, w] * sum_i T_w(t[b, i])

Since |t| < 1 the reference's clip(+-100) is dead code.  We work in the
monomial basis: with power sums M_j[b] = sum_i t[b,i]^j (M_0 = IN_F
exactly) and G = coeffs @ C (C the Chebyshev->monomial matrix),
out = M @ G^T.

Op selection is driven by the cost model's DVE fast-mode table:
  - InstTensorScalarPtr with TWO tensor operands (scalar_tensor_tensor)
    runs 1x, but the scalar-immediate form (tensor_scalar, two-op variant
    with accum_out) runs 4x on fp16 -> a full-row moment extraction is
    ~533ns instead of ~2133ns.
  - tensor_tensor (mult) runs 2x on fp16 -> product streams at ~1067ns.
  - ACT activations (tanh/square) are 1x but a parallel engine, and their
    fused accum_out gives the stream's moment for ~187ns extra.
  - GPSIMD tensor_tensor (walrus-accepted here) adds a third elementwise
    engine at ~2.03ns/col for junk product streams.

Streams: t1=tanh(x), t2=t1^2 (ACT), t3=t2*t1 (DVE), t4=t2^2 (ACT);
junk products t5=t4*t1, t6=t3^2, t7=t4*t3, t8=t4^2 carry M5..M8.  Every
junk stream's columns can be split ACT(square-only)/GPS/DVE per CFG; each
ACT piece accumulates its own m-column, the GPS+DVE columns land in one
junk tile read by a single 4x tensor_scalar accum.  The host duplicates
the matching G rows so the final PE matmul re-merges all pieces.

Per-core layout: data-parallel over batch, 8 cores x 1024 rows,
8 row-tiles of [128, 2048] per core.
"""

import numpy as np

import concourse.bass as bass
import concourse.bacc as bacc
import concourse.mybir as mybir
import concourse.tile as tile
from concourse import masks
from concourse.bass_utils import run_bass_kernel_spmd

# This environment's walrus build rejects raw client-encoded ISA instructions
# ("ISA wrong length" for the 64-byte EVENT_SEMAPHORE_RANGE_CLEAR emitted by
# the TileContext exit barrier).  Replace the range-clear with per-semaphore
# EventSemaphore writes (update_mode=sem-wr-imm, value 0), which this walrus
# accepts, so re-executing the loaded NEFF still sees cleared semaphores.
def _sem_clear_via_events(self, sem_range):
    engines = list(self.bass.engines.values())
    inst = None
    for i, s in enumerate(sem_range):
        eng = engines[i % len(engines)]
        inst = mybir.InstEventSemaphore(
            name=self.bass.get_next_instruction_name(),
            ins=[], outs=[],
            sync_info=mybir.SyncInfo(
                on_wait=[],
                on_update=[mybir.SyncUpdate(
                    sync_type="semaphore", id=s,
                    update_mode="sem-wr-imm", update_value=0,
                )],
            ),
        )
        eng.add_instruction(inst)
    return inst


bass.BassGpSimd.sem_clear = _sem_clear_via_events

N_CORES = 8
BATCH = 8192
IN_F = 2048
OUT_F = 1024
DEG = 8
W = DEG + 1
ROWS_PER_CORE = BATCH // N_CORES  # 1024
P = 128
NTILES = ROWS_PER_CORE // P  # 8

F32 = mybir.dt.float32
F16 = mybir.dt.float16
MULT = mybir.AluOpType.mult
ADD = mybir.AluOpType.add
SQUARE = mybir.ActivationFunctionType.Square
TANH = mybir.ActivationFunctionType.Tanh

# Column splits per junk stream s: (act_cols, gps_cols); DVE takes the rest.
# act pieces are squares so only t6 (=t3^2) and t8 (=t4^2) can use ACT.
CFG = {
    "a2": IN_F,      # ACT cols of t2 square (value stream)
    "a4": IN_F,      # ACT cols of t4 square (value stream)
    "a6": 1024,      # ACT cols of t6 (M6)
    "a8": 0,         # ACT cols of t8 (M8)
    "g5": 2048,      # GPS cols of t5 (M5)
    "g7": 896,       # GPS cols of t7 (M7)
    "g8": 0,         # GPS cols of t8 (M8)
    "oc_act": 0,     # out-copy cols on ACT (rest DVE)
    "warm": True,    # split tile 0's x DMA + tanh into halves
    "tail_delay": 2,  # software-pipeline depth for the per-tile tail
    "mt_act": True,   # moment-transpose PSUM->SBUF copy on ACT
    "gt_act": True,   # gt DMA on the ACT HWDGE queue (SP queue free for x)
    "xin_bufs": 3,
    "val_bufs": 3,
    "junk_bufs": 3,
    "ostage_bufs": 3,
    "tail_fast": True,
}


def _cheb_monomial_matrix(deg=DEG):
    C = np.zeros((deg + 1, deg + 1), dtype=np.float64)
    C[0, 0] = 1.0
    if deg >= 1:
        C[1, 1] = 1.0
    for n in range(2, deg + 1):
        C[n, 1:] = 2.0 * C[n - 1, :-1]
        C[n, :] -= C[n - 2, :]
    return C


def _plan(cfg):
    """Ordered m-column list: (moment_k, tag). Single source of truth for
    both the kernel emission and the host GT row duplication."""
    cols = [(0, "M0"), (1, "M1")]
    if cfg["a2"] > 0:
        cols.append((2, "M2a"))
    if cfg["a2"] < IN_F:
        cols.append((2, "M2d"))
    if cfg["a4"] > 0:
        cols.append((4, "M4a"))
    if cfg["a4"] < IN_F:
        cols.append((4, "M4d"))
    cols.append((3, "M3"))
    for k, a_key, g_key in ((5, None, "g5"), (6, "a6", None),
                            (7, None, "g7"), (8, "a8", "g8")):
        a = cfg.get(a_key, 0) if a_key else 0
        g = cfg.get(g_key, 0) if g_key else 0
        if a > 0:
            cols.append((k, f"M{k}a"))
        if a + g < IN_F or g > 0:
            cols.append((k, f"M{k}d"))
    if cfg.get("warm"):
        cols.append((1, "W1"))
    if cfg.get("warm2"):
        cols.append((2, "W2"))
    return cols


def _build_nc(scale: float, cfg=CFG) -> bass.Bass:
    plan = _plan(cfg)
    K = len(plan)
    assert K <= 32
    idx = {tag: i for i, (_k, tag) in enumerate(plan)}

    nc = bacc.Bacc("TRN2")
    x = nc.dram_tensor("x", [ROWS_PER_CORE, IN_F], F32, kind="ExternalInput")
    gt = nc.dram_tensor("gt", [K, OUT_F], F32, kind="ExternalInput")
    out = nc.dram_tensor("out", [ROWS_PER_CORE, OUT_F], F32,
                         kind="ExternalOutput")

    with tile.TileContext(nc) as tc:
        with (
            tc.tile_pool(name="singles", bufs=1) as singles,
            tc.tile_pool(name="xin", bufs=cfg["xin_bufs"]) as xin,
            tc.tile_pool(name="vals", bufs=cfg["val_bufs"]) as vals,
            tc.tile_pool(name="junk", bufs=cfg["junk_bufs"]) as junkp,
            tc.tile_pool(name="janx", bufs=2) as janx,
            tc.tile_pool(name="mpool", bufs=4) as mpool,
            tc.tile_pool(name="mtsb", bufs=4) as mtsb,
            tc.tile_pool(name="ostage", bufs=cfg["ostage_bufs"]) as ostage,
            tc.tile_pool(name="pt", bufs=2, space="PSUM") as pt,
            tc.tile_pool(name="pout", bufs=2, space="PSUM") as pout,
        ):
            gt_sb = singles.tile([K, OUT_F], F32)
            ident = singles.tile([P, P], F32)
            if not cfg.get("gt_late"):
                # gt on the ACT HWDGE queue keeps the SP queue free for x(0)
                geng = nc.scalar if cfg.get("gt_act") else nc.sync
                geng.dma_start(out=gt_sb[:, :], in_=gt[:, :])
                masks.make_identity(nc, ident[:, :])

            def front(it):
                """Tile front: DMA, tanh, value squares, product streams.
                Returns the closure state for the deferred tail."""
                r0 = it * P
                x_t = xin.tile([P, IN_F], F32)
                chunked = cfg.get("warm") and it == 0
                H = IN_F // 2
                if chunked:
                    nc.sync.dma_start(out=x_t[:, 0:H], in_=x[r0:r0 + P, 0:H])
                    nc.sync.dma_start(out=x_t[:, H:IN_F],
                                      in_=x[r0:r0 + P, H:IN_F])
                else:
                    nc.sync.dma_start(out=x_t[:, :], in_=x[r0:r0 + P, :])
                if it == 0 and cfg.get("gt_late"):
                    # gt + identity after tile 0's x so tanh starts sooner
                    nc.sync.dma_start(out=gt_sb[:, :], in_=gt[:, :])
                    masks.make_identity(nc, ident[:, :])

                m_t = mpool.tile([P, K], F32)
                nc.gpsimd.memset(m_t[:, 0:1], float(IN_F))
                if cfg.get("warm") and not chunked:
                    w = idx["W1"]
                    nc.gpsimd.memset(m_t[:, w:w + 1], 0.0)
                if cfg.get("warm2") and not chunked:
                    w = idx["W2"]
                    nc.gpsimd.memset(m_t[:, w:w + 1], 0.0)

                def mcol(tag):
                    i = idx[tag]
                    return m_t[:, i:i + 1]

                t1 = vals.tile([P, IN_F], F16, tag="t1")
                t2 = vals.tile([P, IN_F], F16, tag="t2")
                t3 = vals.tile([P, IN_F], F16, tag="t3")
                t4 = vals.tile([P, IN_F], F16, tag="t4")

                # t1 = tanh(scale*x), accum -> M1 (warm: halves on tile 0)
                if chunked:
                    nc.scalar.activation(out=t1[:, 0:H], in_=x_t[:, 0:H],
                                         func=TANH, scale=scale,
                                         accum_out=mcol("M1"))
                    nc.scalar.activation(out=t1[:, H:IN_F], in_=x_t[:, H:IN_F],
                                         func=TANH, scale=scale,
                                         accum_out=mcol("W1"))
                else:
                    nc.scalar.activation(out=t1[:, :], in_=x_t[:, :],
                                         func=TANH, scale=scale,
                                         accum_out=mcol("M1"))

                def val_square(src, dst, a_cols, tag_a, tag_d):
                    """Value stream dst = src^2: ACT [0:a] + DVE [a:IN_F]."""
                    if a_cols > 0:
                        nc.scalar.activation(out=dst[:, 0:a_cols],
                                             in_=src[:, 0:a_cols], func=SQUARE,
                                             accum_out=mcol(tag_a))
                    if a_cols < IN_F:
                        nc.vector.tensor_tensor(out=dst[:, a_cols:IN_F],
                                                in0=src[:, a_cols:IN_F],
                                                in1=src[:, a_cols:IN_F],
                                                op=MULT)
                        nc.vector.tensor_scalar(out=dst[:, a_cols:IN_F],
                                                in0=dst[:, a_cols:IN_F],
                                                scalar1=1.0, scalar2=0.0,
                                                op0=MULT, op1=ADD,
                                                accum_out=mcol(tag_d))

                if chunked and cfg.get("warm2"):
                    # cascade tile 0 halves: sq2 + tt3 start one half sooner
                    nc.scalar.activation(out=t2[:, 0:H], in_=t1[:, 0:H],
                                         func=SQUARE, accum_out=mcol("M2a"))
                    nc.scalar.activation(out=t2[:, H:IN_F], in_=t1[:, H:IN_F],
                                         func=SQUARE, accum_out=mcol("W2"))
                    nc.vector.tensor_tensor(out=t3[:, 0:H], in0=t2[:, 0:H],
                                            in1=t1[:, 0:H], op=MULT)
                    nc.vector.tensor_tensor(out=t3[:, H:IN_F],
                                            in0=t2[:, H:IN_F],
                                            in1=t1[:, H:IN_F], op=MULT)
                else:
                    val_square(t1, t2, cfg["a2"], "M2a", "M2d")
                    nc.vector.tensor_tensor(out=t3[:, :], in0=t2[:, :],
                                            in1=t1[:, :], op=MULT)
                tsd = junkp.tile([P, IN_F], F16, tag="tsd")
                nc.vector.tensor_scalar(out=tsd[:, :], in0=t3[:, :],
                                        scalar1=1.0, scalar2=0.0,
                                        op0=MULT, op1=ADD,
                                        accum_out=mcol("M3"))

                val_square(t2, t4, cfg["a4"], "M4a", "M4d")

                deferred = []
                jd_of = {}

                def junk_moment(k, in0, in1, a_cols, g_cols, gps_ins=None):
                    """Junk product stream: ACT square [0:a] (own accum,
                    deferred to the tail since it may wait on DVE's t3),
                    GPS tt [a:a+g], DVE tt [a+g:IN_F]; one 4x ts accum over
                    [a:IN_F], deferred when fed by the slow GPSIMD engine so
                    the in-order DVE queue never stalls on it."""
                    if a_cols > 0:
                        def emit_sq(k=k, in0=in0, a_cols=a_cols):
                            ja = janx.tile([P, a_cols], F16, tag=f"ja{k}")
                            nc.scalar.activation(out=ja[:, :],
                                                 in_=in0[:, 0:a_cols],
                                                 func=SQUARE,
                                                 accum_out=mcol(f"M{k}a"))
                        deferred.append(emit_sq)
                    lo = a_cols
                    mid = min(IN_F, a_cols + g_cols)
                    if lo >= IN_F:
                        return
                    jd = junkp.tile([P, IN_F - lo], F16, tag=f"jd{k}")
                    jd_of[k] = jd
                    if mid > lo:
                        gi0, gi1 = gps_ins if gps_ins else (in0, in1)
                        nc.gpsimd.tensor_tensor(out=jd[:, 0:mid - lo],
                                                in0=gi0[:, lo:mid],
                                                in1=gi1[:, lo:mid], op=MULT)
                    if mid < IN_F:
                        nc.vector.tensor_tensor(out=jd[:, mid - lo:],
                                                in0=in0[:, mid:IN_F],
                                                in1=in1[:, mid:IN_F], op=MULT)

                    def emit_ts(jd=jd, k=k):
                        nc.vector.tensor_scalar(out=jd[:, :], in0=jd[:, :],
                                                scalar1=1.0, scalar2=0.0,
                                                op0=MULT, op1=ADD,
                                                accum_out=mcol(f"M{k}d"))
                    if mid > lo:
                        deferred.append(emit_ts)
                    else:
                        emit_ts()

                last = it == NTILES - 1 and cfg.get("last_no_gps")
                g5 = 0 if last else cfg["g5"]
                g7 = 0 if last else cfg["g7"]
                g8 = 0 if last else cfg["g8"]
                if cfg.get("gps_chain"):
                    # GPS streams avoid t4 (the longest ACT chain): t5 = t2*t3
                    # and t7 = jd5*t2 (Pool self-chain, valid if g7 <= g5).
                    assert g7 <= g5 or g5 == 0
                    junk_moment(5, t2, t3, 0, g5)
                    junk_moment(8, t4, t4, cfg["a8"], g8)
                    junk_moment(6, t3, t3, cfg["a6"], 0)
                    junk_moment(7, t4, t3, 0, g7,
                                gps_ins=(jd_of[5], t2) if g5 else None)
                else:
                    junk_moment(8, t4, t4, cfg["a8"], g8)
                    junk_moment(6, t3, t3, cfg["a6"], 0)
                    junk_moment(5, t4, t1, 0, g5)
                    junk_moment(7, t4, t3, 0, g7)
                return {"it": it, "r0": r0, "m_t": m_t, "deferred": deferred}

            def tail(st):
                """Tile tail, emitted one iteration later so its cross-engine
                waits overlap the next tile's front work."""
                it, r0, m_t = st["it"], st["r0"], st["m_t"]
                for emit in st["deferred"]:
                    emit()

                # Transpose moments [P, K] -> [K, P] PSUM, copy to SBUF
                mt_ps = pt.tile([K, P], F32)
                nc.tensor.transpose(mt_ps[:, :], m_t[:, :], ident[:, :])
                mt_sb = mtsb.tile([K, P], F32)
                if cfg.get("mt_act"):
                    nc.scalar.copy(mt_sb[:, :], mt_ps[:, :])
                else:
                    nc.vector.tensor_copy(mt_sb[:, :], mt_ps[:, :])

                # out[128, 1024] = MT.T @ GT  (contraction K).  f32r runs the
                # PE at 1 cycle/row instead of f32's 4 (same bits, TF32-like
                # precision -- fine at our tolerance).
                F32R = mybir.dt.float32r
                lhs_ap = mt_sb[:, :]
                o_ps = pout.tile([P, OUT_F], F32)
                for h in range(2):
                    rhs_ap = gt_sb[:, h * 512:(h + 1) * 512]
                    if cfg.get("f32r"):
                        nc.tensor.matmul(o_ps[:, h * 512:(h + 1) * 512],
                                         lhsT=lhs_ap.bitcast(F32R),
                                         rhs=rhs_ap.bitcast(F32R),
                                         start=True, stop=True)
                    else:
                        nc.tensor.matmul(o_ps[:, h * 512:(h + 1) * 512],
                                         lhsT=lhs_ap, rhs=rhs_ap,
                                         start=True, stop=True)
                o_sb = ostage.tile([P, OUT_F], F32)
                ca = cfg["oc_act"]
                if cfg.get("tail_fast") and it == NTILES - 1:
                    H2 = OUT_F // 2
                    nc.scalar.copy(o_sb[:, 0:H2], o_ps[:, 0:H2])
                    nc.vector.tensor_copy(o_sb[:, H2:OUT_F], o_ps[:, H2:OUT_F])
                    nc.sync.dma_start(out=out[r0:r0 + P, 0:H2],
                                      in_=o_sb[:, 0:H2])
                    nc.sync.dma_start(out=out[r0:r0 + P, H2:OUT_F],
                                      in_=o_sb[:, H2:OUT_F])
                    return
                if ca > 0:
                    nc.scalar.copy(o_sb[:, 0:ca], o_ps[:, 0:ca])
                if ca < OUT_F:
                    nc.vector.tensor_copy(o_sb[:, ca:OUT_F], o_ps[:, ca:OUT_F])
                nc.sync.dma_start(out=out[r0:r0 + P, :], in_=o_sb[:, :])

            pending = []
            D = cfg.get("tail_delay", 1)
            for it in range(NTILES):
                st = front(it)
                pending.append(st)
                if len(pending) > D:
                    tail(pending.pop(0))
            for st in pending:
                tail(st)

    nc.finalize()
    return nc


_NC_CACHE: dict[tuple, bass.Bass] = {}


def _host_gt(coeffs, cfg=CFG):
    C = _cheb_monomial_matrix()
    G = (coeffs.astype(np.float64) @ C).astype(np.float32)  # [OUT_F, W]
    rows = [k for k, _tag in _plan(cfg)]
    GT = np.ascontiguousarray(G.T[rows, :])  # [K, OUT_F]
    return GT


def _run(x, coeffs, input_scale, cfg=CFG, **spmd_kwargs):
    x = np.ascontiguousarray(np.asarray(x, dtype=np.float32))
    coeffs = np.asarray(coeffs, dtype=np.float32)
    scale = float(np.clip(np.asarray(input_scale, dtype=np.float32),
                          0.1, 2.0).reshape(-1)[0])

    GT = _host_gt(coeffs, cfg)

    key = (scale, str(cfg))
    nc = _NC_CACHE.get(key)
    if nc is None:
        nc = _build_nc(scale, cfg)
        _NC_CACHE[key] = nc

    in_maps = [
        {"x": np.ascontiguousarray(x[c * ROWS_PER_CORE:(c + 1) * ROWS_PER_CORE]),
         "gt": GT}
        for c in range(N_CORES)
    ]
    res = run_bass_kernel_spmd(nc, in_maps, core_ids=list(range(N_CORES)),
                               **spmd_kwargs)
    out = np.concatenate([res.results[c]["out"] for c in range(N_CORES)],
                         axis=0)
    return out.astype(np.float32), res


def kernel(x, coeffs, input_scale):
    out, _ = _run(x, coeffs, input_scale)
    return out


if __name__ == "__main__":
    rng = np.random.default_rng(0)
    x = rng.standard_normal((BATCH, IN_F), dtype=np.float32)
    coeffs = (rng.standard_normal((OUT_F, W)) * 0.1).astype(np.float32)
    s = np.ones((1,), np.float32)
    out = kernel(x=x, coeffs=coeffs, input_scale=s)
    print(out.shape, out.dtype)
